# revision 34
# baseline (speedup 1.0000x reference)
"""ChameleonAttention Trainium2 kernel (v5).

Full-input contract: kernel(**inputs) with the complete tensors; internally
shards tensor-parallel across 8 NeuronCores by attention head (4 heads/core):
  - w_qkv rows + q/k norm params sharded by head
  - w_o columns sharded by head, partial outputs summed on host (all-reduce)

Host pre-transposes and pre-converts the big operands to bf16 (hidden^T,
w_qkv^T, w_o^T) so tiles DMA straight into matmul-ready layouts.

v5 scheduling (vs v4):
  - hidden-tile DMAs issued before the w_qkv DMA; the first two token groups
    run k-chunk-major (6 PSUM chains in flight) so the PE chases the weight
    DMA instead of idling until all of wT lands.
  - q/k PE-transposes emitted one group late so they never head-of-line
    block the next group's matmul chains.
  - softmax denominators via DVE f32 accumulation of the exp tiles + one
    gpsimd partition_all_reduce per (block, head) - no per-tile ones-matmul
    on PE, no reciprocal broadcast.
  - PV matmuls sliced to the causal width; no E-prefix memsets.
  - o_proj for block b-1 interleaved unit-by-unit into block b's attention
    i-loop, filling the PE gaps left by the exp latency; output DMA per
    128-token row tile.
"""
import sys

sys.path.insert(0, "/opt/trn_rl_repo")

import numpy as np
import ml_dtypes

import concourse.bass as bass
import concourse.mybir as mybir
import concourse.tile as tile
from concourse import bacc
from concourse import bass_isa
from concourse.bass_utils import run_bass_kernel_spmd
from concourse.masks import make_identity, make_upper_triangular

P = 128
T = 2048
HID = 4096
D = 128
HPC = 4  # heads per core
R = 3 * HPC  # 12 qkv row-tiles per core
NK = HID // P  # 32 k-tiles
NTT = T // P  # 16 token tiles
THETA = 10000.0
EPS = 1e-5
SCALE = D ** -0.5
TWO_PI = 6.283185307179586
C_HI = float(np.float32(6.28125))
C_LO = TWO_PI - C_HI

f32 = mybir.dt.float32
bf16 = mybir.dt.bfloat16
i32 = mybir.dt.int32
AF = mybir.ActivationFunctionType
ALU = mybir.AluOpType
RED = bass_isa.ReduceOp

_NC_CACHE = {}


def build_nc():
    nc = bacc.Bacc(None, target_bir_lowering=False, debug=False)

    # hidden pre-tiled on host: element (tg, p, kk, t) = hidden[tg*128+t,
    # kk*128+p] so each partition's slice of a token-group tile is one
    # contiguous 8 KB run (vs 256 B lines from a plain [HID, T] transpose).
    hiddenT = nc.dram_tensor("hiddenT", (NTT, P, NK, P), bf16,
                             kind="ExternalInput")
    wqT = nc.dram_tensor("wqT", (HID, R * P), bf16, kind="ExternalInput")
    woT = nc.dram_tensor("woT", (HPC * D, HID), bf16, kind="ExternalInput")
    pos = nc.dram_tensor("pos", (T,), i32, kind="ExternalInput")
    qnw = nc.dram_tensor("qnw", (HPC, D), bf16, kind="ExternalInput")
    qnb = nc.dram_tensor("qnb", (HPC, D), bf16, kind="ExternalInput")
    knw = nc.dram_tensor("knw", (HPC, D), bf16, kind="ExternalInput")
    knb = nc.dram_tensor("knb", (HPC, D), bf16, kind="ExternalInput")
    out = nc.dram_tensor("out", (T, HID), bf16, kind="ExternalOutput")

    with tile.TileContext(nc) as tc:
        with tc.tile_pool(name="const", bufs=1) as const:
            p1_scope = tc.tile_pool(name="p1_wT", bufs=1)
            p_wT = p1_scope.__enter__()
            p1_scope_h = tc.tile_pool(name="p1_hT", bufs=3)
            p_hT = p1_scope_h.__enter__()
            # --- hidden tiles first (PE prerequisite), then the big wT ---
            hTs = {}

            def fetch_h(tg):
                hT = p_hT.tile([P, NK, P], bf16, tag="hT")
                nc.sync.dma_start(hT[:], hiddenT[tg])
                hTs[tg] = hT
                return hT

            wT = p_wT.tile([P, NK, R * P], bf16, tag="wT")
            wsrc = wqT.rearrange("(kk p) c -> p kk c", p=P)

            def fetch_w(kg):
                nc.sync.dma_start(wT[:, kg * 4 : (kg + 1) * 4, :],
                                  wsrc[:, kg * 4 : (kg + 1) * 4, :])

            # small tensors whose DMAs must not sit behind the wT bulk:
            # pos feeds the rope-table chain, nrm/nrm2 feed the LN affine
            pos_i = const.tile([P, NTT], i32)
            _pre_cm = tc.tile_pool(name="pre", bufs=1)
            pre = _pre_cm.__enter__()
            nrm = pre.tile([1, 8, D], bf16)
            nrm2 = pre.tile([1, 8, D], bf16)

            def fetch_h_part(hT, tg, kg):
                nc.sync.dma_start(
                    hT[:, kg * 4 : (kg + 1) * 4, :],
                    hiddenT[tg, :, kg * 4 : (kg + 1) * 4, :])

            # DMA queue order tuned so the PE can start ASAP and is then
            # paced by the wT chunks: the two boot hidden tiles stream
            # kg-chunked alongside the matching wT chunk.
            hb0 = p_hT.tile([P, NK, P], bf16, tag="hT", name="hb0")
            hb1 = p_hT.tile([P, NK, P], bf16, tag="hT", name="hb1")
            hb2 = p_hT.tile([P, NK, P], bf16, tag="hT", name="hb2")
            hTs[0], hTs[1], hTs[2] = hb0, hb1, hb2
            fetch_h_part(hb0, 0, 0)
            fetch_w(0)
            fetch_h_part(hb1, 1, 0)
            fetch_h_part(hb2, 2, 0)
            for kg in range(1, 8):
                fetch_h_part(hb0, 0, kg)
                fetch_h_part(hb1, 1, kg)
                fetch_h_part(hb2, 2, kg)
                fetch_w(kg)
                if kg == 1:
                    nc.sync.dma_start(pos_i[:],
                                      pos.rearrange("(i p) -> p i", p=P))
                    nc.sync.dma_start(nrm[:, 0:4, :],
                                      qnw.rearrange("h d -> (h d)"))
                    nc.sync.dma_start(nrm[:, 4:8, :],
                                      knw.rearrange("h d -> (h d)"))
                    nc.sync.dma_start(nrm2[:, 0:4, :],
                                      qnb.rearrange("h d -> (h d)"))
                    nc.sync.dma_start(nrm2[:, 4:8, :],
                                      knb.rearrange("h d -> (h d)"))

            # --- constants ---
            ident_b = const.tile([P, P], bf16)
            triu_b = const.tile([P, P], bf16)
            epsc = const.tile([P, 1], f32)
            nc.vector.memset(epsc[:], EPS)
            w8 = const.tile([P, 8, D], bf16)  # LN weight, bcast over t
            b8 = const.tile([P, 8, D], bf16)
            c4 = const.tile([P, NTT, HPC, 64], bf16)  # cos, replicated x4 heads
            s4 = const.tile([P, NTT, HPC, 64], bf16)
            qkT = const.tile([P, 8, T], bf16)  # post-rope q(0:4), k(4:8); [d, t]
            v_nat = const.tile([P, NTT, 512], bf16)  # [t, 4 heads * d]

            with tc.tile_pool(name="cstage", bufs=1) as cstage:
                ident_f = cstage.tile([P, P], f32)
                make_identity(nc, ident_f[:])
                nc.vector.tensor_copy(ident_b[:], ident_f[:])
                triu_f = cstage.tile([P, P], f32)
                make_upper_triangular(nc, triu_f[:], val=1.0, diag=True)
                nc.vector.tensor_copy(triu_b[:], triu_f[:])

                # LN affine params broadcast over partitions: w8/b8[p, g*4+h, :]
                nc.gpsimd.partition_broadcast(w8[:], nrm[:])
                nc.gpsimd.partition_broadcast(b8[:], nrm2[:])
            _pre_cm.__exit__(None, None, None)

            # --- rope tables in natural layout: [t(part), tt, 64] ---
            with tc.tile_pool(name="rtmp", bufs=1) as rtmp:
                jj = rtmp.tile([1, 64], f32)
                nc.gpsimd.iota(jj[:], pattern=[[1, 64]], base=0,
                               channel_multiplier=0,
                               allow_small_or_imprecise_dtypes=True)
                invf = rtmp.tile([1, 64], f32)
                nc.scalar.activation(invf[:], jj[:], AF.Exp,
                                     scale=-float(np.log(THETA)) / 64.0)
                invf_b = rtmp.tile([P, 64], f32)
                nc.gpsimd.partition_broadcast(invf_b[:], invf[:])
                # t values from positions input: tval[p, i] = pos[i*128 + p]
                tval = rtmp.tile([P, NTT], f32)
                nc.vector.tensor_copy(tval[:], pos_i[:])
                freqs = rtmp.tile([P, NTT, 64], f32)
                for i in range(NTT):
                    nc.vector.tensor_scalar_mul(freqs[:, i, :], invf_b[:],
                                                tval[:, i : i + 1])

                HT2 = NTT // 2

                def reduced_sin(dst_ap, src_ap):
                    # dst = sin(reduce(src)), reduce(x) = x - 2pi*round(x/2pi)
                    q = rtmp.tile([P, HT2, 64], f32, tag="rs_q")
                    nc.vector.tensor_scalar_mul(q[:], src_ap, 1.0 / TWO_PI)
                    n_i = rtmp.tile([P, HT2, 64], i32, tag="rs_n")
                    nc.vector.tensor_copy(n_i[:], q[:])  # round-to-nearest
                    n_f = rtmp.tile([P, HT2, 64], f32, tag="rs_nf")
                    nc.vector.tensor_copy(n_f[:], n_i[:])
                    r0 = rtmp.tile([P, HT2, 64], f32, tag="rs_r0")
                    nc.vector.scalar_tensor_tensor(
                        out=r0[:], in0=n_f[:], scalar=-C_HI, in1=src_ap,
                        op0=ALU.mult, op1=ALU.add)
                    r1 = rtmp.tile([P, HT2, 64], f32, tag="rs_r1")
                    nc.vector.scalar_tensor_tensor(
                        out=r1[:], in0=n_f[:], scalar=-C_LO, in1=r0[:],
                        op0=ALU.mult, op1=ALU.add)
                    nc.scalar.activation(dst_ap, r1[:], AF.Sin)

                for hh in range(2):
                    tsl = slice(hh * HT2, (hh + 1) * HT2)
                    sc1 = rtmp.tile([P, HT2, 64], bf16, tag="sc1")
                    reduced_sin(sc1[:], freqs[:, tsl, :])
                    for h in range(HPC):
                        nc.vector.tensor_copy(s4[:, tsl, h, :], sc1[:])
                    fr2 = rtmp.tile([P, HT2, 64], f32, tag="fr2")
                    nc.vector.tensor_scalar_add(fr2[:], freqs[:, tsl, :],
                                                np.pi / 2)
                    sc2 = rtmp.tile([P, HT2, 64], bf16, tag="sc2")
                    reduced_sin(sc2[:], fr2[:])
                    for h in range(HPC):
                        nc.vector.tensor_copy(c4[:, tsl, h, :], sc2[:])

            # ---------------- P1 + LN + RoPE ----------
            _qkv_cm = tc.tile_pool(name="p1_qkv", bufs=3)
            p_qkv = _qkv_cm.__enter__()
            _scr_cm = tc.tile_pool(name="p1_scr", bufs=2)
            p_scr = _scr_cm.__enter__()
            _ro_cm = tc.tile_pool(name="p1_ro", bufs=3)
            p_ro = _ro_cm.__enter__()
            _st_cm = tc.tile_pool(name="p1_st", bufs=2)
            p_st = _st_cm.__enter__()
            ps_tp = None  # created after the boot scope (PSUM bank budget)

            def post_pe(tg, qkv):
                """LN + affine + rope for token group tg (no PE work).
                Returns a closure emitting the PE transposes."""
                nc.scalar.activation(v_nat[:, tg, :],
                                     qkv[:, 2 * 512 : 3 * 512], AF.Copy)
                ros = []
                for grp in range(2):  # 0: q heads, 1: k heads
                    xg = qkv[:, grp * 512 : (grp + 1) * 512]
                    xg4 = xg.rearrange("p (h d) -> p h d", h=HPC)
                    s1 = p_st.tile([P, HPC], f32, tag="s1")
                    s2 = p_st.tile([P, HPC], f32, tag="s2")
                    sqs = p_scr.tile([P, 512], bf16, tag="sqs")
                    nc.scalar.activation(sqs[:], xg, AF.Square)
                    nc.vector.tensor_reduce(
                        s2[:], sqs.rearrange("p (h d) -> p h d", h=HPC),
                        axis=mybir.AxisListType.X, op=ALU.add)
                    nc.vector.tensor_reduce(
                        s1[:], xg4, axis=mybir.AxisListType.X, op=ALU.add)
                    mu = p_st.tile([P, HPC], f32, tag="mu")
                    nc.vector.tensor_scalar_mul(mu[:], s1[:], 1.0 / D)
                    musq = p_st.tile([P, HPC], f32, tag="musq")
                    nc.vector.tensor_mul(musq[:], mu[:], mu[:])
                    varv = p_st.tile([P, HPC], f32, tag="varv")
                    nc.vector.scalar_tensor_tensor(
                        out=varv[:], in0=s2[:], scalar=1.0 / D,
                        in1=musq[:], op0=ALU.mult, op1=ALU.subtract)
                    stdv = p_st.tile([P, HPC], f32, tag="stdv")
                    nc.scalar.activation(stdv[:], varv[:], AF.Sqrt,
                                         bias=epsc[:])
                    rstd = p_st.tile([P, HPC], f32, tag="rstd")
                    nc.vector.reciprocal(rstd[:], stdv[:])
                    y = p_scr.tile([P, 512], bf16, tag="y")
                    for h in range(HPC):
                        nc.vector.tensor_scalar(
                            out=y[:, h * D : (h + 1) * D],
                            in0=xg4[:, h, :],
                            scalar1=mu[:, h : h + 1],
                            scalar2=rstd[:, h : h + 1],
                            op0=ALU.subtract, op1=ALU.mult)
                    y4 = y.rearrange("p (h d) -> p h d", h=HPC)
                    nc.vector.tensor_mul(y4, y4, w8[:, grp * 4 : grp * 4 + 4, :])
                    nc.vector.tensor_add(y4, y4, b8[:, grp * 4 : grp * 4 + 4, :])
                    # rope: halves along d
                    yh = y.rearrange("p (h v d) -> p h v d", h=HPC, v=2)
                    ro = p_ro.tile([P, 512], bf16, tag="ro")
                    roh = ro.rearrange("p (h v d) -> p h v d", h=HPC, v=2)
                    tmp = p_scr.tile([P, 512], bf16, tag="tmp")
                    tmph = tmp.rearrange("p (h v d) -> p h v d", h=HPC, v=2)
                    cc = c4[:, tg, :, :]
                    ss = s4[:, tg, :, :]
                    nc.vector.tensor_mul(tmph[:, :, 0, :], yh[:, :, 0, :], cc)
                    nc.vector.tensor_mul(tmph[:, :, 1, :], yh[:, :, 1, :], ss)
                    nc.vector.tensor_sub(roh[:, :, 0, :], tmph[:, :, 0, :],
                                         tmph[:, :, 1, :])
                    nc.vector.tensor_mul(tmph[:, :, 0, :], yh[:, :, 1, :], cc)
                    nc.vector.tensor_mul(tmph[:, :, 1, :], yh[:, :, 0, :], ss)
                    nc.vector.tensor_add(roh[:, :, 1, :], tmph[:, :, 0, :],
                                         tmph[:, :, 1, :])
                    ros.append(ro)

                def emit_transposes():
                    ta = tg * P
                    for grp in range(2):
                        ro = ros[grp]
                        pq = ps_tp.tile([P, 4, P], bf16, tag="tpb")
                        for h in range(HPC):
                            nc.tensor.transpose(
                                pq[:, h, :], ro[:, h * D : (h + 1) * D],
                                ident_b[:])
                        nc.scalar.activation(
                            qkT[:, grp * 4 : grp * 4 + 4, ta : ta + P], pq[:],
                            AF.Copy)

                return emit_transposes

            pending_tp = []

            def drain_tp(all=False):
                if all:
                    while pending_tp:
                        pending_tp.pop(0)()
                elif pending_tp:
                    pending_tp.pop(0)()

            # --- boot: groups 0,1 k-chunk-major so PE chases the wT DMA ---
            BOOT = [(0, 0), (0, 1), (0, 2), (1, 0), (1, 1), (1, 2),
                    (2, 0), (2, 1)]
            qkv2 = None
            with tc.tile_pool(name="ps_boot", bufs=1, space="PSUM") as ps_boot:
                accs = {}
                for g, cb in BOOT:
                    accs[(g, cb)] = ps_boot.tile(
                        [P, 512], f32, tag=f"boot{g}{cb}", name=f"boot{g}{cb}")
                for kg in range(8):
                    for g, cb in BOOT:
                        for kk in range(kg * 4, kg * 4 + 4):
                            nc.tensor.matmul(
                                accs[(g, cb)][:],
                                hTs[g][:, kk, :],
                                wT[:, kk, cb * 512 : (cb + 1) * 512],
                                start=(kk == 0), stop=(kk == NK - 1))
                # all boot copies first - they gate PSUM bank reuse by the
                # following chains, so they must not queue behind LN work.
                # Alternate ACT/DVE to halve the serial latency. (gpsimd
                # cannot read PSUM on hardware.)
                qkvs = {}
                for g in range(2):
                    qkvs[g] = p_qkv.tile([P, R * P], bf16, tag="qkv",
                                         name=f"qkvb{g}")
                qkv2 = p_qkv.tile([P, R * P], bf16, tag="qkv", name="qkv2")
                qkvs[2] = qkv2
                for n, (g, cb) in enumerate(BOOT):
                    dst = qkvs[g][:, cb * 512 : (cb + 1) * 512]
                    if n % 2 == 0:
                        nc.scalar.activation(dst, accs[(g, cb)][:], AF.Copy)
                    else:
                        nc.vector.tensor_copy(dst, accs[(g, cb)][:])
                for g in range(2):
                    pending_tp.append(post_pe(g, qkvs[g]))
            _tp_cm = tc.tile_pool(name="ps_tp", bufs=2, space="PSUM")
            ps_tp = _tp_cm.__enter__()

            with tc.tile_pool(name="ps_acc", bufs=2, space="PSUM") as ps_acc:
                for tg in range(2, NTT):
                    ta = tg * P
                    if tg + 1 < NTT and (tg + 1) not in hTs:
                        fetch_h(tg + 1)
                    hT = hTs[tg]
                    if tg == 2:
                        qkv = qkv2
                        cbs = [2]
                    else:
                        qkv = p_qkv.tile([P, R * P], bf16, tag="qkv")
                        cbs = [0, 1, 2]
                    for cb in cbs:
                        acc = ps_acc.tile([P, 512], f32, tag=f"acc{cb}")
                        for kk in range(NK):
                            nc.tensor.matmul(
                                acc[:],
                                hT[:, kk, :],
                                wT[:, kk, cb * 512 : (cb + 1) * 512],
                                start=(kk == 0), stop=(kk == NK - 1))
                        nc.scalar.activation(qkv[:, cb * 512 : (cb + 1) * 512],
                                             acc[:], AF.Copy)
                    drain_tp()
                    pending_tp.append(post_pe(tg, qkv))
                drain_tp(all=True)

            # preload the Exp activation table before the first real exp
            nc.scalar.activation(epsc[:, 0:1], epsc[:, 0:1], AF.Exp)
            _tp_cm.__exit__(None, None, None)
            _st_cm.__exit__(None, None, None)
            _ro_cm.__exit__(None, None, None)
            _scr_cm.__exit__(None, None, None)
            _qkv_cm.__exit__(None, None, None)
            p1_scope_h.__exit__(None, None, None)
            p1_scope.__exit__(None, None, None)

            # -------- P2 + P3 interleaved --------
            with (
                tc.tile_pool(name="p2_attnT", bufs=1) as p_attnT,
                tc.tile_pool(name="p3_wo", bufs=1) as p3_wo,
            ):
                attnT = p_attnT.tile([P, HPC, T], bf16, tag="attnT")
                wo_all = p3_wo.tile([P, 8, 4, 512], bf16, tag="wo_all")
                wos = woT.rearrange("(c p) (nb n) -> p nb c n", p=P, n=512)
                for nb in range(8):
                    nc.sync.dma_start(wo_all[:, nb, :, :], wos[:, nb, :, :])
                with (
                    tc.tile_pool(name="p2_E", bufs=8) as p_E,
                    tc.tile_pool(name="p2_es", bufs=1) as p_es,
                    tc.tile_pool(name="p2_tail", bufs=2) as p_tail,
                    tc.tile_pool(name="p3_o", bufs=3) as p3_o,
                    tc.tile_pool(name="ps_st", bufs=4, space="PSUM") as ps_st,
                    tc.tile_pool(name="ps_at", bufs=1, space="PSUM") as ps_at,
                    tc.tile_pool(name="ps_o", bufs=2, space="PSUM") as ps_o,
                ):
                    def oproj_units(b):
                        """Generator of o_proj emit-closures for q-block b,
                        one unit per (nb, 128-token row tile j)."""
                        for nb in range(8):
                            for j in range(4):
                                def unit(nb=nb, j=j):
                                    t = 4 * b + j
                                    po = ps_o.tile([P, 512], f32, tag="po")
                                    for c in range(4):
                                        nc.tensor.matmul(
                                            po[:],
                                            attnT[:, c, t * P : (t + 1) * P],
                                            wo_all[:, nb, c, :],
                                            start=(c == 0), stop=(c == 3))
                                    o_sb = p3_o.tile([P, 512], bf16, tag="o_sb")
                                    nc.vector.tensor_copy(o_sb[:], po[:])
                                    nc.sync.dma_start(
                                        out[t * P : (t + 1) * P,
                                            nb * 512 : (nb + 1) * 512],
                                        o_sb[:])
                                yield unit

                    pending_oproj = []  # closures from block b-1

                    for b in range(4):
                        bs = slice(b * 512, (b + 1) * 512)
                        n_i = 4 * b + 4
                        # heads run round-robin so each head's exp latency is
                        # hidden behind the other heads' matmuls; o_proj units
                        # from block b-1 fill the rest (skip the first round
                        # so attnT(b-1) writes have drained).
                        oq = pending_oproj
                        rounds = max(1, 2 * n_i - 4)
                        per_round = -(-len(oq) // rounds) if oq else 0
                        for pair in ((0, 1), (2, 3)):
                            at_ps = {}
                            esum = {}
                            for h in pair:
                                at_ps[h] = ps_at.tile(
                                    [P, 512], f32, tag=f"at{h % 2}",
                                    name=f"at{h}")
                                esum[h] = p_es.tile(
                                    [P, 512], f32, tag=f"es{h % 2}",
                                    name=f"es{h}")
                            for i in range(n_i):
                                moff = max(0, (i - 4 * b) * P)
                                nv = 512 - moff
                                Es = {}
                                # all score matmuls + exps first, then the
                                # PV pair - so the PE never sits behind an
                                # exp it could have prefetched around
                                for h in pair:
                                    st = ps_st.tile([P, 512], f32, tag="st")
                                    nc.tensor.matmul(
                                        st[:, 0:nv],
                                        qkT[:, 4 + h, i * P : (i + 1) * P],
                                        qkT[:, h, b * 512 + moff : (b + 1) * 512],
                                        start=True, stop=True)
                                    E = p_E.tile([P, 512], bf16, tag="E")
                                    nc.scalar.activation(
                                        E[:, moff:512], st[:, 0:nv],
                                        AF.Exp, scale=SCALE)
                                    if i >= 4 * b:
                                        nc.vector.tensor_mul(
                                            E[:, moff : moff + P],
                                            E[:, moff : moff + P], triu_b[:])
                                    if i == 0:
                                        nc.gpsimd.tensor_copy(esum[h][:], E[:])
                                    else:
                                        nc.gpsimd.tensor_add(
                                            esum[h][:, moff:512],
                                            esum[h][:, moff:512],
                                            E[:, moff:512])
                                    Es[h] = E
                                for h in pair:
                                    nc.tensor.matmul(
                                        at_ps[h][:, moff:512],
                                        v_nat[:, i, h * D : (h + 1) * D],
                                        Es[h][:, moff:512],
                                        start=(i == 0), stop=(i == n_i - 1))
                                if i >= 2 or (i >= 1 and pair[0] == 2):
                                    for _ in range(per_round):
                                        if oq:
                                            oq.pop(0)()
                            for h in pair:
                                # tail: denominator + normalize (no PE)
                                den = p_tail.tile([P, 512], f32, tag="den")
                                nc.gpsimd.partition_all_reduce(
                                    den[:], esum[h][:], 128, RED.add)
                                rc = p_tail.tile([P, 512], f32, tag="rc")
                                nc.vector.reciprocal_approx_fast(out=rc[:],
                                                                 in_=den[:])
                                nc.vector.tensor_mul(attnT[:, h, bs],
                                                     at_ps[h][:], rc[:])
                        while oq:
                            oq.pop(0)()
                        pending_oproj = list(oproj_units(b))
                    for u in pending_oproj:
                        u()

    nc.compile()
    return nc


def _get_nc():
    if "nc" not in _NC_CACHE:
        _NC_CACHE["nc"] = build_nc()
    return _NC_CACHE["nc"]


def _shard_inputs(positions, hidden_states, w_qkv, w_o, q_norm_w, q_norm_b,
                  k_norm_w, k_norm_b):
    H = 32
    bf = ml_dtypes.bfloat16
    hsb = np.asarray(hidden_states, np.float32).astype(bf)
    hT = np.ascontiguousarray(
        hsb.reshape(16, 128, 32, 128).transpose(0, 3, 2, 1))
    pos_np = np.ascontiguousarray(positions, dtype=np.int32)
    in_maps = []
    for c in range(8):
        hs = slice(c * HPC, (c + 1) * HPC)
        rows = np.concatenate(
            [
                w_qkv[c * 512 : (c + 1) * 512],
                w_qkv[H * D + c * 512 : H * D + (c + 1) * 512],
                w_qkv[2 * H * D + c * 512 : 2 * H * D + (c + 1) * 512],
            ],
            axis=0,
        )
        in_maps.append(
            {
                "hiddenT": hT,
                "wqT": np.ascontiguousarray(
                    np.asarray(rows, np.float32).T.astype(bf)),
                "woT": np.ascontiguousarray(
                    np.asarray(w_o[:, c * 512 : (c + 1) * 512], np.float32)
                    .T.astype(bf)),
                "pos": pos_np,
                "qnw": np.ascontiguousarray(
                    np.asarray(q_norm_w[hs], np.float32).astype(bf)),
                "qnb": np.ascontiguousarray(
                    np.asarray(q_norm_b[hs], np.float32).astype(bf)),
                "knw": np.ascontiguousarray(
                    np.asarray(k_norm_w[hs], np.float32).astype(bf)),
                "knb": np.ascontiguousarray(
                    np.asarray(k_norm_b[hs], np.float32).astype(bf)),
            }
        )
    return in_maps


def kernel(positions, hidden_states, w_qkv, w_o, q_norm_w, q_norm_b,
           k_norm_w, k_norm_b):
    nc = _get_nc()
    in_maps = _shard_inputs(
        np.asarray(positions), np.asarray(hidden_states), np.asarray(w_qkv),
        np.asarray(w_o), np.asarray(q_norm_w), np.asarray(q_norm_b),
        np.asarray(k_norm_w), np.asarray(k_norm_b),
    )
    res = run_bass_kernel_spmd(nc, in_maps, list(range(8))).results
    acc = np.zeros((T, HID), np.float64)
    for c in range(8):
        acc += res[c]["out"].astype(np.float64)
    return acc.astype(np.float32)


if __name__ == "__main__":
    build_nc()
    print("build OK")
